# revision 13
# baseline (speedup 1.0000x reference)
"""Trainium2 Bass kernel for nn_GCNNet (gnn_message_passing).

Structure exploited: the batch graph has B=512 graphs of exactly NPG=128
nodes each (nodes per graph == SBUF partition width), and every edge stays
within its source node's graph.  So per graph g the GCN aggregation
  out = D^-1/2 (A+I) D^-1/2 @ Z
is a dense 128x128 matmul.  Host preprocessing only converts the edge list
into per-graph uint8 adjacency *counts* (index bookkeeping); all float math
(degrees, normalization, 3 GCN layers, dense MLP head, global max pool +
pooled MLP head) runs on device.

Sharding: graph-parallel.  Core c owns graphs [c*64, (c+1)*64) -> its slice
of nodes/edges/outputs.  Weights are replicated.  No cross-core comms.

Layout scheme (zero on-device data transposes): alternate per layer between
natural [node, feat] and transposed [feat, node] activations:
  pattern A (nat -> T):   Ut = lhsT=H_nat  rhs=AtN     (=(Â H)^T)
                          Ht' = lhsT=W     rhs=Ut      (=W^T (ÂH)^T = (ÂHW)^T)
  pattern B (T -> nat):   Z  = lhsT=Ht     rhs=W       (=H W)
                          H' = lhsT=AtN   rhs=Z       (=Â Z)
where AtN[s,d] = dis[s]*(A+I)[d,s]*dis[d] is the *transposed* normalized
adjacency (exactly what the lhsT/rhs slots want).  AtN is built from the
natural counts A[d,s]: row-scale by dis (per-partition), transpose on the
PE against an identity, scale by dis again on the PSUM->SBUF copy.

Graphs run in blocks of BG=4 (512-wide moving operands amortize the weight
load; elementwise ops run once per block).  Emission is software-pipelined:
each block's adjacency prologue (DMA, cast, degree, dis, row-scale,
PE-transpose, normalize) is emitted PIPE_AHEAD blocks before its body, so
the in-order engine queues precompute the next blocks' adjacencies while
the current body runs — removing block-boundary bubbles on the PE.
The final head matmul runs transposed (stationary fW2) producing
out_t[g, o, n]; the host transposes to [g, n, o] while unsharding.
"""

import numpy as np
import ml_dtypes

import concourse.bacc as bacc
import concourse.mybir as mybir
from concourse.tile import TileContext
from concourse.bass_utils import run_bass_kernel_spmd

# Problem constants (hardcoded per the harness contract).
N, B, F, E = 65536, 512, 64, 1048576
NPG = 128                 # nodes per graph == partition width
NCORES = 8
GPC = B // NCORES         # 64 graphs per core
BG = 4                    # graphs per block
BW = BG * NPG             # 512: block width in the free dim
PIPE_AHEAD = 2            # prologue emission lead (blocks)

F2 = 2 * F                # 128
F4 = 4 * F                # 256
FH = 1024                 # head hidden
FO = 128                  # head out

f32 = mybir.dt.float32
u8 = mybir.dt.uint8

AluOp = mybir.AluOpType
ActFn = mybir.ActivationFunctionType
AxX = mybir.AxisListType.X


def build_program(dt=mybir.dt.bfloat16, gpc=GPC):
    """Build + compile the per-core Bass program (SPMD, identical per core)."""
    assert gpc % BG == 0
    nblk = gpc // BG
    nc = bacc.Bacc("TRN2", target_bir_lowering=False, debug=False)

    x_d = nc.declare_dram_parameter("x", [gpc, NPG, F], dt, isOutput=False)
    # natural adjacency counts incl self loops: adj[g, d, s] = #(s->d) + I
    adj_d = nc.declare_dram_parameter("adj", [gpc, NPG, NPG], dt, isOutput=False)
    ident_d = nc.declare_dram_parameter("ident", [NPG, NPG], dt, isOutput=False)
    w1_d = nc.declare_dram_parameter("w1", [F, F], dt, isOutput=False)
    w2_d = nc.declare_dram_parameter("w2", [F, F2], dt, isOutput=False)
    w3_d = nc.declare_dram_parameter("w3", [F2, F4], dt, isOutput=False)
    fw1_d = nc.declare_dram_parameter("fw1", [2, 128, FH], dt, isOutput=False)
    fw2_d = nc.declare_dram_parameter("fw2", [8, 128, FO], dt, isOutput=False)
    b1_d = nc.declare_dram_parameter("b1", [F], f32, isOutput=False)
    b2x_d = nc.declare_dram_parameter("b2x", [BG * F2], f32, isOutput=False)
    b3_d = nc.declare_dram_parameter("b3", [2, 128], f32, isOutput=False)
    fb1_d = nc.declare_dram_parameter("fb1", [8, 128], f32, isOutput=False)
    fb2_d = nc.declare_dram_parameter("fb2", [FO], f32, isOutput=False)
    # transposed dense-head output: out_t[g, o, n] (host swaps o/n axes)
    outt_d = nc.declare_dram_parameter("out_t", [gpc, FO, NPG], f32, isOutput=True)
    outp_d = nc.declare_dram_parameter("out_pool", [gpc, FO], f32, isOutput=True)

    with TileContext(nc) as tc:
        with (
            tc.tile_pool(name="const", bufs=1) as cp,
            tc.tile_pool(name="work", bufs=3) as wp,
            tc.tile_pool(name="pro", bufs=PIPE_AHEAD + 2) as prp,
            tc.tile_pool(name="ys", bufs=2) as ys_pool,
            tc.tile_pool(name="psStage", bufs=3, space="PSUM") as ppS,
            tc.tile_pool(name="psAtp", bufs=2, space="PSUM") as ppT,
            tc.tile_pool(name="psY", bufs=2, space="PSUM") as ppY,
            tc.tile_pool(name="psO", bufs=1, space="PSUM") as ppO,
        ):
            # ---- persistent weights / constants ----
            w1s = cp.tile([F, F], dt)
            nc.sync.dma_start(out=w1s[:], in_=w1_d[:])
            w2s = cp.tile([F, F2], dt)
            nc.sync.dma_start(out=w2s[:], in_=w2_d[:])
            w3s = cp.tile([F2, F4], dt)
            nc.sync.dma_start(out=w3s[:], in_=w3_d[:])
            fw1s = cp.tile([128, 2, FH], dt)
            nc.sync.dma_start(out=fw1s[:], in_=fw1_d[:].transpose([1, 0, 2]))
            fw2s = cp.tile([128, 8, FO], dt)
            nc.sync.dma_start(out=fw2s[:], in_=fw2_d[:].transpose([1, 0, 2]))
            idents = cp.tile([NPG, NPG], dt)
            nc.sync.dma_start(out=idents[:], in_=ident_d[:])
            b1s = cp.tile([F, 1], f32)
            nc.sync.dma_start(out=b1s[:], in_=b1_d[:].unsqueeze(1))
            b2xs = cp.tile([1, BG * F2], f32)
            nc.sync.dma_start(out=b2xs[:], in_=b2x_d[:].unsqueeze(0))
            b3s = cp.tile([128, 2], f32)
            nc.sync.dma_start(out=b3s[:], in_=b3_d[:].transpose([1, 0]))
            fb1s = cp.tile([128, 8], f32)
            nc.sync.dma_start(out=fb1s[:], in_=fb1_d[:].transpose([1, 0]))
            fb2s = cp.tile([1, FO], f32)
            nc.sync.dma_start(out=fb2s[:], in_=fb2_d[:].unsqueeze(0))
            fb2c = cp.tile([FO, 1], f32)   # per-partition column for out2t bias
            nc.sync.dma_start(out=fb2c[:], in_=fb2_d[:].unsqueeze(1))
            ones_row = cp.tile([1, NPG], f32)
            nc.vector.memset(ones_row[:], 1.0)
            # materialized free-dim bias broadcasts (built once via K=1 matmul)
            b2bc_ps = ppT.tile([NPG, BG * F2], f32, tag="atp")
            nc.tensor.matmul(b2bc_ps[:], lhsT=ones_row[:], rhs=b2xs[:],
                             start=True, stop=True)
            b2bc = cp.tile([NPG, BG * F2], f32)
            nc.scalar.copy(b2bc[:], b2bc_ps[:])
            fb2bc_ps = ppT.tile([NPG, FO], f32, tag="atp")
            nc.tensor.matmul(fb2bc_ps[:], lhsT=ones_row[:], rhs=fb2s[:],
                             start=True, stop=True)
            fb2bc = cp.tile([NPG, FO], f32)
            nc.scalar.copy(fb2bc[:], fb2bc_ps[:])
            # pooled per-graph maxes, feature-major (filled per block)
            pool_a = cp.tile([128, gpc], dt)
            pool_b = cp.tile([128, gpc], dt)

            pend = {}

            def prologue(blk):
                g0 = blk * BG
                xb = prp.tile([NPG, BG, F], dt, tag="xb")
                nc.sync.dma_start(out=xb[:],
                                  in_=x_d[g0:g0 + BG].transpose([1, 0, 2]))
                adjb = prp.tile([NPG, BG, NPG], dt, tag="adj8")
                nc.sync.dma_start(out=adjb[:],
                                  in_=adj_d[g0:g0 + BG].transpose([1, 0, 2]))
                # degrees + D^-1/2 for the whole block
                degb = prp.tile([NPG, BG], f32, tag="degb")
                nc.vector.reduce_sum(degb[:], adjb[:], axis=AxX)
                rdegb = prp.tile([NPG, BG], f32, tag="rdegb")
                nc.vector.reciprocal(rdegb[:], degb[:])
                disb = prp.tile([NPG, BG], f32, tag="disb")
                nc.scalar.sqrt(disb[:], rdegb[:])
                # normalized transposed adjacency AtN (block-merged scalings)
                acrb = prp.tile([NPG, BG, NPG], dt, tag="acr")
                nc.vector.tensor_tensor(
                    out=acrb[:], in0=adjb[:],
                    in1=disb[:].unsqueeze(2).broadcast_to([NPG, BG, NPG]),
                    op=AluOp.mult)
                atpb = ppT.tile([NPG, BG, NPG], f32, tag="atp")
                for j in range(BG):
                    nc.tensor.matmul(atpb[:, j, :], lhsT=acrb[:, j, :],
                                     rhs=idents[:], start=True, stop=True,
                                     skip_group_check=True)
                atnb = prp.tile([NPG, BG, NPG], dt, tag="atn")
                nc.vector.tensor_tensor(
                    out=atnb[:], in0=atpb[:],
                    in1=disb[:].unsqueeze(2).broadcast_to([NPG, BG, NPG]),
                    op=AluOp.mult)
                pend[blk] = (xb, atnb)

            pend_h3 = {}

            def gcn_gen(blk):
                g0 = blk * BG
                xb, atnb = pend.pop(blk)
                atns = [atnb[:, j, :] for j in range(BG)]

                # L1 (pattern A): X nat -> H1t
                u1 = ppS.tile([F, BG, NPG], f32, tag="stage")
                for j in range(BG):
                    nc.tensor.matmul(u1[:, j, :], lhsT=xb[:, j, :], rhs=atns[j],
                                     start=True, stop=True, skip_group_check=True)
                yield
                u1s = wp.tile([F, BG * NPG], dt, tag="u1s")
                nc.vector.tensor_copy(u1s[:], u1[:])
                h1p = ppS.tile([F, BW], f32, tag="stage")
                nc.tensor.matmul(h1p[:], lhsT=w1s[:], rhs=u1s[:],
                                 start=True, stop=True)
                h1 = wp.tile([F, BW], dt, tag="h1")
                nc.scalar.activation(h1[:], h1p[:], ActFn.Relu, bias=b1s[:],
                                     scale=1.0)
                yield

                # L2 (pattern B): H1t -> H2 nat
                z2 = ppS.tile([NPG, BG, F2], f32, tag="stage")
                for j in range(BG):
                    nc.tensor.matmul(z2[:, j, :],
                                     lhsT=h1[:, j * NPG:(j + 1) * NPG],
                                     rhs=w2s[:], start=True, stop=True,
                                     skip_group_check=True)
                yield
                z2s = wp.tile([NPG, BG * F2], dt, tag="z2s")
                nc.vector.tensor_copy(z2s[:], z2[:])
                h2p = ppS.tile([NPG, BG, F2], f32, tag="stage")
                for j in range(BG):
                    nc.tensor.matmul(h2p[:, j, :], lhsT=atns[j],
                                     rhs=z2s[:, j * F2:(j + 1) * F2],
                                     start=True, stop=True, skip_group_check=True)
                h2b = wp.tile([NPG, BG * F2], f32, tag="h2b")
                nc.vector.tensor_tensor(out=h2b[:], in0=h2p[:], in1=b2bc[:],
                                        op=AluOp.add)
                h2 = wp.tile([NPG, BG * F2], dt, tag="h2")
                nc.scalar.activation(h2[:], h2b[:], ActFn.Relu)
                yield

                # L3 (pattern A): H2 nat -> H3t (256 feats = 2 tiles)
                u3 = ppS.tile([F2, BG, NPG], f32, tag="stage")
                for j in range(BG):
                    nc.tensor.matmul(u3[:, j, :],
                                     lhsT=h2[:, j * F2:(j + 1) * F2],
                                     rhs=atns[j], start=True, stop=True,
                                     skip_group_check=True)
                yield
                u3s = wp.tile([F2, BW], dt, tag="u3s")
                nc.scalar.copy(u3s[:], u3[:])
                h3pa = ppS.tile([128, BW], f32, tag="stage")
                nc.tensor.matmul(h3pa[:], lhsT=w3s[:, 0:128], rhs=u3s[:],
                                 start=True, stop=True)
                yield
                h3pb = ppS.tile([128, BW], f32, tag="stage")
                nc.tensor.matmul(h3pb[:], lhsT=w3s[:, 128:256], rhs=u3s[:],
                                 start=True, stop=True)
                h3a = wp.tile([128, BW], dt, tag="h3a")
                nc.scalar.activation(h3a[:], h3pa[:], ActFn.Relu,
                                     bias=b3s[:, 0:1], scale=1.0)
                h3b = wp.tile([128, BW], dt, tag="h3b")
                nc.scalar.activation(h3b[:], h3pb[:], ActFn.Relu,
                                     bias=b3s[:, 1:2], scale=1.0)
                yield

                # global max pool (over nodes = innermost free dim)
                nc.vector.reduce_max(pool_a[:, g0:g0 + BG],
                                     h3a[:].rearrange("p (g n) -> p g n", g=BG),
                                     axis=AxX)
                nc.vector.reduce_max(pool_b[:, g0:g0 + BG],
                                     h3b[:].rearrange("p (g n) -> p g n", g=BG),
                                     axis=AxX)
                pend_h3[blk] = (h3a, h3b)

            def head_gen(blk):
                h3a, h3b = pend_h3.pop(blk)
                g0 = blk * BG
                # dense head: Y1t then transposed out2
                o2t = ppO.tile([FO, BW], f32, tag="o2")
                ys_tiles = []
                for t in range(8):
                    ypp = ppY.tile([128, BW], f32, tag="yp")
                    nc.tensor.matmul(ypp[:],
                                     lhsT=fw1s[:, 0, t * 128:(t + 1) * 128],
                                     rhs=h3a[:], start=True, stop=False)
                    nc.tensor.matmul(ypp[:],
                                     lhsT=fw1s[:, 1, t * 128:(t + 1) * 128],
                                     rhs=h3b[:], start=False, stop=True)
                    ys = ys_pool.tile([128, BW], dt, tag=f"ys{t}")
                    if t in (1, 4, 6):
                        nc.vector.tensor_scalar(
                            out=ys[:], in0=ypp[:], scalar1=fb1s[:, t:t + 1],
                            scalar2=0.0, op0=AluOp.add, op1=AluOp.max)
                    else:
                        nc.scalar.activation(ys[:], ypp[:], ActFn.Relu,
                                             bias=fb1s[:, t:t + 1], scale=1.0)
                    ys_tiles.append(ys)
                    # interleave out2t accumulation (lag 3 so the relu can land)
                    if t >= 3:
                        nc.tensor.matmul(o2t[:], lhsT=fw2s[:, t - 3, :],
                                         rhs=ys_tiles[t - 3][:],
                                         start=(t == 3), stop=False)
                    yield
                for t in (5, 6, 7):
                    nc.tensor.matmul(o2t[:], lhsT=fw2s[:, t, :],
                                     rhs=ys_tiles[t][:],
                                     start=False, stop=(t == 7))
                o2st = wp.tile([FO, BW], f32, tag="o2st")
                nc.vector.tensor_scalar(out=o2st[:], in0=o2t[:], scalar1=fb2c[:],
                                        scalar2=None, op0=AluOp.add)
                nc.sync.dma_start(
                    out=outt_d[g0:g0 + BG].transpose([1, 0, 2]),
                    in_=o2st[:].rearrange("p (g n) -> p g n", g=BG))

            # ---- pooled head (once per core, all graphs batched) ----
            def pool_head_gen():
                # yp tag (not o2): the last block's o2t still holds the o2 slot
                o2p = ppY.tile([gpc, FO], f32, tag="yp")
                yps_tiles = []
                for t in range(8):
                    ypp = ppY.tile([128, gpc], f32, tag="yp")
                    nc.tensor.matmul(ypp[:],
                                     lhsT=fw1s[:, 0, t * 128:(t + 1) * 128],
                                     rhs=pool_a[:], start=True, stop=False)
                    nc.tensor.matmul(ypp[:],
                                     lhsT=fw1s[:, 1, t * 128:(t + 1) * 128],
                                     rhs=pool_b[:], start=False, stop=True)
                    yps = ys_pool.tile([128, gpc], dt, tag=f"ys{t}")
                    if t in (1, 4, 6):
                        nc.vector.tensor_scalar(
                            out=yps[:], in0=ypp[:], scalar1=fb1s[:, t:t + 1],
                            scalar2=0.0, op0=AluOp.add, op1=AluOp.max)
                    else:
                        nc.scalar.activation(yps[:], ypp[:], ActFn.Relu,
                                             bias=fb1s[:, t:t + 1], scale=1.0)
                    yps_tiles.append(yps)
                    if t >= 1:
                        nc.tensor.matmul(o2p[:], lhsT=yps_tiles[t - 1][:],
                                         rhs=fw2s[:, t - 1, :],
                                         start=(t == 1), stop=False)
                    yield
                nc.tensor.matmul(o2p[:], lhsT=yps_tiles[7][:], rhs=fw2s[:, 7, :],
                                 start=False, stop=True)
                o2ps = wp.tile([gpc, FO], f32, tag="o2ps")
                nc.vector.tensor_tensor(out=o2ps[:], in0=o2p[:],
                                        in1=fb2bc[0:gpc, :], op=AluOp.add)
                nc.sync.dma_start(out=outp_d[:], in_=o2ps[:])

            # software-pipelined emission: prologues run PIPE_AHEAD blocks
            # early; block b's head stages zip with block b+1's GCN stages so
            # every cross-engine handoff has independent PE work queued behind
            # it (in-order engine queues).
            def drain(*gens):
                gens = [g for g in gens if g is not None]
                while gens:
                    alive = []
                    for g in gens:
                        try:
                            next(g)
                            alive.append(g)
                        except StopIteration:
                            pass
                    gens = alive

            prologue(0)
            if nblk > 1:
                prologue(1)
            drain(gcn_gen(0))
            for blk in range(nblk):
                if blk + 2 < nblk:
                    prologue(blk + 2)
                drain(head_gen(blk),
                      gcn_gen(blk + 1) if blk + 1 < nblk else None,
                      pool_head_gen() if blk == nblk - 1 else None)


    nc.compile()
    return nc


def _np_dt(dt):
    return ml_dtypes.bfloat16 if dt == mybir.dt.bfloat16 else np.float32


def host_prep(x, src, dst, W1, b1, W2, b2, W3, b3, fW1, fb1, fW2, fb2,
              dt=mybir.dt.bfloat16):
    """Host-side index bookkeeping + per-core sharding. Returns in_maps."""
    ndt = _np_dt(dt)
    src = np.asarray(src).astype(np.int64)
    dst = np.asarray(dst).astype(np.int64)

    # Natural per-graph adjacency counts adj[g, d, s] = #(edges s->d) + I.
    g = src >> 7
    cell = (g << 14) | ((dst & 127) << 7) | (src & 127)
    cnt = np.bincount(cell, minlength=B * NPG * NPG)
    diag = ((np.arange(B, dtype=np.int64) << 14)[:, None]
            + (np.arange(NPG, dtype=np.int64) * (NPG + 1))[None, :]).ravel()
    cnt[diag] += 1
    assert cnt.max() < 256, "adjacency count overflow"
    adj = cnt.astype(ndt).reshape(B, NPG, NPG)

    x = np.asarray(x, dtype=np.float32).astype(ndt).reshape(B, NPG, F)
    common = dict(
        ident=np.eye(NPG, dtype=np.float32).astype(ndt),
        w1=np.asarray(W1, np.float32).astype(ndt),
        w2=np.asarray(W2, np.float32).astype(ndt),
        w3=np.asarray(W3, np.float32).astype(ndt),
        fw1=np.asarray(fW1, np.float32).astype(ndt).reshape(2, 128, FH),
        fw2=np.asarray(fW2, np.float32).astype(ndt).reshape(8, 128, FO),
        b1=np.asarray(b1, np.float32),
        b2x=np.tile(np.asarray(b2, np.float32), BG),
        b3=np.asarray(b3, np.float32).reshape(2, 128),
        fb1=np.asarray(fb1, np.float32).reshape(8, 128),
        fb2=np.asarray(fb2, np.float32),
    )
    in_maps = []
    for c in range(NCORES):
        in_maps.append(dict(
            x=np.ascontiguousarray(x[c * GPC:(c + 1) * GPC]),
            adj=np.ascontiguousarray(adj[c * GPC:(c + 1) * GPC]),
            **common,
        ))
    return in_maps


_compiled = {}


def _get_program(dt):
    key = str(dt)
    if key not in _compiled:
        _compiled[key] = build_program(dt=dt)
    return _compiled[key]


def kernel(x, src, dst, batch, W1, b1, W2, b2, W3, b3, fW1, fb1, fW2, fb2,
           dt=mybir.dt.bfloat16):
    # `batch` is the deterministic repeat(arange(B), NPG) — structure hardcoded.
    in_maps = host_prep(x, src, dst, W1, b1, W2, b2, W3, b3, fW1, fb1, fW2, fb2,
                        dt=dt)
    nc = _get_program(dt)
    res = run_bass_kernel_spmd(nc, in_maps, list(range(NCORES)))
    outs = res.results
    out_t = np.concatenate([r["out_t"] for r in outs], axis=0)    # [B, o, n]
    p = np.concatenate([r["out_pool"] for r in outs], axis=0)
    out = np.ascontiguousarray(out_t.transpose(0, 2, 1))          # [B, n, o]
    return out, p.reshape(B, FO)


# revision 15
# speedup vs baseline: 1.0458x; 1.0458x over previous
"""Trainium2 Bass kernel for nn_GCNNet (gnn_message_passing).

Structure exploited: the batch graph has B=512 graphs of exactly NPG=128
nodes each (nodes per graph == SBUF partition width), and every edge stays
within its source node's graph.  So per graph g the GCN aggregation
  out = D^-1/2 (A+I) D^-1/2 @ Z
is a dense 128x128 matmul.  Host preprocessing only converts the edge list
into per-graph uint8 adjacency *counts* (index bookkeeping); all float math
(degrees, normalization, 3 GCN layers, dense MLP head, global max pool +
pooled MLP head) runs on device.

Sharding: graph-parallel.  Core c owns graphs [c*64, (c+1)*64) -> its slice
of nodes/edges/outputs.  Weights are replicated.  No cross-core comms.

Layout scheme (zero on-device data transposes): alternate per layer between
natural [node, feat] and transposed [feat, node] activations:
  pattern A (nat -> T):   Ut = lhsT=H_nat  rhs=AtN     (=(Â H)^T)
                          Ht' = lhsT=W     rhs=Ut      (=W^T (ÂH)^T = (ÂHW)^T)
  pattern B (T -> nat):   Z  = lhsT=Ht     rhs=W       (=H W)
                          H' = lhsT=AtN   rhs=Z       (=Â Z)
where AtN[s,d] = dis[s]*(A+I)[d,s]*dis[d] is the *transposed* normalized
adjacency (exactly what the lhsT/rhs slots want).  AtN is built from the
natural counts A[d,s]: row-scale by dis (per-partition), transpose on the
PE against an identity, scale by dis again on the PSUM->SBUF copy.

Graphs run in blocks of BG=4 (512-wide moving operands amortize the weight
load; elementwise ops run once per block).  Emission is software-pipelined:
each block's adjacency prologue (DMA, cast, degree, dis, row-scale,
PE-transpose, normalize) is emitted PIPE_AHEAD blocks before its body, so
the in-order engine queues precompute the next blocks' adjacencies while
the current body runs — removing block-boundary bubbles on the PE.
The final head matmul runs transposed (stationary fW2) producing
out_t[g, o, n]; the host transposes to [g, n, o] while unsharding.
"""

import numpy as np
import ml_dtypes

import concourse.bacc as bacc
import concourse.mybir as mybir
from concourse.tile import TileContext
from concourse.bass_utils import run_bass_kernel_spmd

# Problem constants (hardcoded per the harness contract).
N, B, F, E = 65536, 512, 64, 1048576
NPG = 128                 # nodes per graph == partition width
NCORES = 8
GPC = B // NCORES         # 64 graphs per core
BG = 4                    # graphs per block
BW = BG * NPG             # 512: block width in the free dim
PIPE_AHEAD = 2            # prologue emission lead (blocks)

F2 = 2 * F                # 128
F4 = 4 * F                # 256
FH = 1024                 # head hidden
FO = 128                  # head out

f32 = mybir.dt.float32
u8 = mybir.dt.uint8

AluOp = mybir.AluOpType
ActFn = mybir.ActivationFunctionType
AxX = mybir.AxisListType.X


def build_program(dt=mybir.dt.bfloat16, gpc=GPC):
    """Build + compile the per-core Bass program (SPMD, identical per core)."""
    assert gpc % BG == 0
    nblk = gpc // BG
    nc = bacc.Bacc("TRN2", target_bir_lowering=False, debug=False)

    x_d = nc.declare_dram_parameter("x", [gpc, NPG, F], dt, isOutput=False)
    # natural adjacency counts incl self loops: adj[g, d, s] = #(s->d) + I
    adj_d = nc.declare_dram_parameter("adj", [gpc, NPG, NPG], dt, isOutput=False)
    ident_d = nc.declare_dram_parameter("ident", [NPG, NPG], dt, isOutput=False)
    w1_d = nc.declare_dram_parameter("w1", [F, F], dt, isOutput=False)
    w2_d = nc.declare_dram_parameter("w2", [F, F2], dt, isOutput=False)
    w3_d = nc.declare_dram_parameter("w3", [F2, F4], dt, isOutput=False)
    fw1_d = nc.declare_dram_parameter("fw1", [2, 128, FH], dt, isOutput=False)
    fw2_d = nc.declare_dram_parameter("fw2", [8, 128, FO], dt, isOutput=False)
    b1_d = nc.declare_dram_parameter("b1", [F], f32, isOutput=False)
    b2x_d = nc.declare_dram_parameter("b2x", [BG * F2], f32, isOutput=False)
    b3_d = nc.declare_dram_parameter("b3", [2, 128], f32, isOutput=False)
    fb1_d = nc.declare_dram_parameter("fb1", [8, 128], f32, isOutput=False)
    fb2_d = nc.declare_dram_parameter("fb2", [FO], f32, isOutput=False)
    # transposed dense-head output: out_t[g, o, n] (host swaps o/n axes)
    outt_d = nc.declare_dram_parameter("out_t", [gpc, FO, NPG], f32, isOutput=True)
    outp_d = nc.declare_dram_parameter("out_pool", [gpc, FO], f32, isOutput=True)

    with TileContext(nc) as tc:
        with (
            tc.tile_pool(name="const", bufs=1) as cp,
            tc.tile_pool(name="work", bufs=3) as wp,
            tc.tile_pool(name="pro", bufs=PIPE_AHEAD + 2) as prp,
            tc.tile_pool(name="ys", bufs=2) as ys_pool,
            tc.tile_pool(name="psStage", bufs=3, space="PSUM") as ppS,
            tc.tile_pool(name="psAtp", bufs=2, space="PSUM") as ppT,
            tc.tile_pool(name="psY", bufs=2, space="PSUM") as ppY,
            tc.tile_pool(name="psO", bufs=1, space="PSUM") as ppO,
        ):
            # ---- persistent weights / constants ----
            idents = cp.tile([NPG, NPG], dt)
            nc.sync.dma_start(out=idents[:], in_=ident_d[:])
            b1s = cp.tile([F, 1], f32)
            nc.sync.dma_start(out=b1s[:], in_=b1_d[:].unsqueeze(1))
            b2xs = cp.tile([1, BG * F2], f32)
            nc.sync.dma_start(out=b2xs[:], in_=b2x_d[:].unsqueeze(0))
            b3s = cp.tile([128, 2], f32)
            nc.sync.dma_start(out=b3s[:], in_=b3_d[:].transpose([1, 0]))
            fb1s = cp.tile([128, 8], f32)
            nc.sync.dma_start(out=fb1s[:], in_=fb1_d[:].transpose([1, 0]))
            fb2s = cp.tile([1, FO], f32)
            nc.sync.dma_start(out=fb2s[:], in_=fb2_d[:].unsqueeze(0))
            fb2c = cp.tile([FO, 1], f32)   # per-partition column for out2t bias
            nc.sync.dma_start(out=fb2c[:], in_=fb2_d[:].unsqueeze(1))
            ones_row = cp.tile([1, NPG], f32)
            nc.vector.memset(ones_row[:], 1.0)
            # materialized free-dim bias broadcasts (built once via K=1 matmul)
            b2bc_ps = ppT.tile([NPG, BG * F2], f32, tag="atp")
            nc.tensor.matmul(b2bc_ps[:], lhsT=ones_row[:], rhs=b2xs[:],
                             start=True, stop=True)
            b2bc = cp.tile([NPG, BG * F2], f32)
            nc.scalar.copy(b2bc[:], b2bc_ps[:])
            fb2bc_ps = ppT.tile([NPG, FO], f32, tag="atp")
            nc.tensor.matmul(fb2bc_ps[:], lhsT=ones_row[:], rhs=fb2s[:],
                             start=True, stop=True)
            fb2bc = cp.tile([NPG, FO], f32)
            nc.scalar.copy(fb2bc[:], fb2bc_ps[:])
            # pooled per-graph maxes, feature-major (filled per block)
            pool_a = cp.tile([128, gpc], dt)
            pool_b = cp.tile([128, gpc], dt)

            pend = {}

            def prologue(blk):
                g0 = blk * BG
                xb = prp.tile([NPG, BG, F], dt, tag="xb")
                nc.sync.dma_start(out=xb[:],
                                  in_=x_d[g0:g0 + BG].transpose([1, 0, 2]))
                adjb = prp.tile([NPG, BG, NPG], dt, tag="adj8")
                nc.sync.dma_start(out=adjb[:],
                                  in_=adj_d[g0:g0 + BG].transpose([1, 0, 2]))
                # degrees + D^-1/2 for the whole block
                degb = prp.tile([NPG, BG], f32, tag="degb")
                nc.vector.reduce_sum(degb[:], adjb[:], axis=AxX)
                rdegb = prp.tile([NPG, BG], f32, tag="rdegb")
                nc.vector.reciprocal(rdegb[:], degb[:])
                disb = prp.tile([NPG, BG], f32, tag="disb")
                nc.scalar.sqrt(disb[:], rdegb[:])
                # normalized transposed adjacency AtN (block-merged scalings)
                acrb = prp.tile([NPG, BG, NPG], dt, tag="acr")
                nc.vector.tensor_tensor(
                    out=acrb[:], in0=adjb[:],
                    in1=disb[:].unsqueeze(2).broadcast_to([NPG, BG, NPG]),
                    op=AluOp.mult)
                atpb = ppT.tile([NPG, BG, NPG], f32, tag="atp")
                for j in range(BG):
                    nc.tensor.matmul(atpb[:, j, :], lhsT=acrb[:, j, :],
                                     rhs=idents[:], start=True, stop=True,
                                     skip_group_check=True)
                atnb = prp.tile([NPG, BG, NPG], dt, tag="atn")
                nc.vector.tensor_tensor(
                    out=atnb[:], in0=atpb[:],
                    in1=disb[:].unsqueeze(2).broadcast_to([NPG, BG, NPG]),
                    op=AluOp.mult)
                pend[blk] = (xb, atnb)

            pend_h3 = {}

            def gcn_gen(blk):
                g0 = blk * BG
                xb, atnb = pend.pop(blk)
                atns = [atnb[:, j, :] for j in range(BG)]

                # L1 (pattern A): X nat -> H1t
                u1 = ppS.tile([F, BG, NPG], f32, tag="stage")
                for j in range(BG):
                    nc.tensor.matmul(u1[:, j, :], lhsT=xb[:, j, :], rhs=atns[j],
                                     start=True, stop=True, skip_group_check=True)
                yield
                u1s = wp.tile([F, BG * NPG], dt, tag="u1s")
                nc.vector.tensor_copy(u1s[:], u1[:])
                h1p = ppS.tile([F, BW], f32, tag="stage")
                nc.tensor.matmul(h1p[:], lhsT=w1s[:], rhs=u1s[:],
                                 start=True, stop=True)
                h1 = wp.tile([F, BW], dt, tag="h1")
                nc.scalar.activation(h1[:], h1p[:], ActFn.Relu, bias=b1s[:],
                                     scale=1.0)
                yield

                # L2 (pattern B): H1t -> H2 nat
                z2 = ppS.tile([NPG, BG, F2], f32, tag="stage")
                for j in range(BG):
                    nc.tensor.matmul(z2[:, j, :],
                                     lhsT=h1[:, j * NPG:(j + 1) * NPG],
                                     rhs=w2s[:], start=True, stop=True,
                                     skip_group_check=True)
                yield
                z2s = wp.tile([NPG, BG * F2], dt, tag="z2s")
                nc.vector.tensor_copy(z2s[:], z2[:])
                h2p = ppS.tile([NPG, BG, F2], f32, tag="stage")
                for j in range(BG):
                    nc.tensor.matmul(h2p[:, j, :], lhsT=atns[j],
                                     rhs=z2s[:, j * F2:(j + 1) * F2],
                                     start=True, stop=True, skip_group_check=True)
                h2b = wp.tile([NPG, BG * F2], f32, tag="h2b")
                nc.vector.tensor_tensor(out=h2b[:], in0=h2p[:], in1=b2bc[:],
                                        op=AluOp.add)
                h2 = wp.tile([NPG, BG * F2], dt, tag="h2")
                nc.scalar.activation(h2[:], h2b[:], ActFn.Relu)
                yield

                # L3 (pattern A): H2 nat -> H3t (256 feats = 2 tiles)
                u3 = ppS.tile([F2, BG, NPG], f32, tag="stage")
                for j in range(BG):
                    nc.tensor.matmul(u3[:, j, :],
                                     lhsT=h2[:, j * F2:(j + 1) * F2],
                                     rhs=atns[j], start=True, stop=True,
                                     skip_group_check=True)
                yield
                u3s = wp.tile([F2, BW], dt, tag="u3s")
                nc.scalar.copy(u3s[:], u3[:])
                h3pa = ppS.tile([128, BW], f32, tag="stage")
                nc.tensor.matmul(h3pa[:], lhsT=w3s[:, 0:128], rhs=u3s[:],
                                 start=True, stop=True)
                yield
                h3pb = ppS.tile([128, BW], f32, tag="stage")
                nc.tensor.matmul(h3pb[:], lhsT=w3s[:, 128:256], rhs=u3s[:],
                                 start=True, stop=True)
                h3a = wp.tile([128, BW], dt, tag="h3a")
                nc.scalar.activation(h3a[:], h3pa[:], ActFn.Relu,
                                     bias=b3s[:, 0:1], scale=1.0)
                h3b = wp.tile([128, BW], dt, tag="h3b")
                nc.scalar.activation(h3b[:], h3pb[:], ActFn.Relu,
                                     bias=b3s[:, 1:2], scale=1.0)
                yield

                # global max pool (over nodes = innermost free dim)
                nc.vector.reduce_max(pool_a[:, g0:g0 + BG],
                                     h3a[:].rearrange("p (g n) -> p g n", g=BG),
                                     axis=AxX)
                nc.vector.reduce_max(pool_b[:, g0:g0 + BG],
                                     h3b[:].rearrange("p (g n) -> p g n", g=BG),
                                     axis=AxX)
                pend_h3[blk] = (h3a, h3b)

            def head_gen(blk):
                h3a, h3b = pend_h3.pop(blk)
                g0 = blk * BG
                # dense head: Y1t then transposed out2
                o2t = ppO.tile([FO, BW], f32, tag="o2")
                ys_tiles = []
                for t in range(8):
                    ypp = ppY.tile([128, BW], f32, tag="yp")
                    nc.tensor.matmul(ypp[:],
                                     lhsT=fw1s[:, 0, t * 128:(t + 1) * 128],
                                     rhs=h3a[:], start=True, stop=False)
                    nc.tensor.matmul(ypp[:],
                                     lhsT=fw1s[:, 1, t * 128:(t + 1) * 128],
                                     rhs=h3b[:], start=False, stop=True)
                    ys = ys_pool.tile([128, BW], dt, tag=f"ys{t}")
                    if t in (1, 4, 6):
                        nc.vector.tensor_scalar(
                            out=ys[:], in0=ypp[:], scalar1=fb1s[:, t:t + 1],
                            scalar2=0.0, op0=AluOp.add, op1=AluOp.max)
                    else:
                        nc.scalar.activation(ys[:], ypp[:], ActFn.Relu,
                                             bias=fb1s[:, t:t + 1], scale=1.0)
                    ys_tiles.append(ys)
                    # interleave out2t accumulation (lag 2 so the relu can land)
                    if t >= 2:
                        nc.tensor.matmul(o2t[:], lhsT=fw2s[:, t - 2, :],
                                         rhs=ys_tiles[t - 2][:],
                                         start=(t == 2), stop=False)
                    yield
                for t in (6, 7):
                    nc.tensor.matmul(o2t[:], lhsT=fw2s[:, t, :],
                                     rhs=ys_tiles[t][:],
                                     start=False, stop=(t == 7))
                o2st = wp.tile([FO, BW], f32, tag="o2st")
                nc.scalar.add(o2st[:], o2t[:], add=fb2c[:])
                nc.sync.dma_start(
                    out=outt_d[g0:g0 + BG].transpose([1, 0, 2]),
                    in_=o2st[:].rearrange("p (g n) -> p g n", g=BG))

            def pool_head_gen():
                # yp tag (not o2): the last block's o2t still holds the o2 slot
                o2p = ppY.tile([gpc, FO], f32, tag="yp")
                yps_tiles = []
                for t in range(8):
                    ypp = ppY.tile([128, gpc], f32, tag="yp")
                    nc.tensor.matmul(ypp[:],
                                     lhsT=fw1s[:, 0, t * 128:(t + 1) * 128],
                                     rhs=pool_a[:], start=True, stop=False)
                    nc.tensor.matmul(ypp[:],
                                     lhsT=fw1s[:, 1, t * 128:(t + 1) * 128],
                                     rhs=pool_b[:], start=False, stop=True)
                    yps = ys_pool.tile([128, gpc], dt, tag=f"ys{t}")
                    if t in (1, 4, 6):
                        nc.vector.tensor_scalar(
                            out=yps[:], in0=ypp[:], scalar1=fb1s[:, t:t + 1],
                            scalar2=0.0, op0=AluOp.add, op1=AluOp.max)
                    else:
                        nc.scalar.activation(yps[:], ypp[:], ActFn.Relu,
                                             bias=fb1s[:, t:t + 1], scale=1.0)
                    yps_tiles.append(yps)
                    if t >= 1:
                        nc.tensor.matmul(o2p[:], lhsT=yps_tiles[t - 1][:],
                                         rhs=fw2s[:, t - 1, :],
                                         start=(t == 1), stop=False)
                    yield
                nc.tensor.matmul(o2p[:], lhsT=yps_tiles[7][:], rhs=fw2s[:, 7, :],
                                 start=False, stop=True)
                o2ps = wp.tile([gpc, FO], f32, tag="o2ps")
                nc.vector.tensor_tensor(out=o2ps[:], in0=o2p[:],
                                        in1=fb2bc[0:gpc, :], op=AluOp.add)
                nc.sync.dma_start(out=outp_d[:], in_=o2ps[:])

            # software-pipelined emission: prologues run PIPE_AHEAD blocks
            # early; block b's head stages zip with block b+1's GCN stages so
            # every cross-engine handoff has independent PE work queued behind
            # it (in-order engine queues).
            def drain(*gens):
                gens = [g for g in gens if g is not None]
                while gens:
                    alive = []
                    for g in gens:
                        try:
                            next(g)
                            alive.append(g)
                        except StopIteration:
                            pass
                    gens = alive

            prologue(0)
            if nblk > 1:
                prologue(1)
            # big weight DMAs queue AFTER the first blocks' adj/x DMAs so the
            # adjacency chain (and PE) starts immediately
            w1s = cp.tile([F, F], dt)
            nc.sync.dma_start(out=w1s[:], in_=w1_d[:])
            w2s = cp.tile([F, F2], dt)
            nc.sync.dma_start(out=w2s[:], in_=w2_d[:])
            w3s = cp.tile([F2, F4], dt)
            nc.sync.dma_start(out=w3s[:], in_=w3_d[:])
            fw1s = cp.tile([128, 2, FH], dt)
            nc.sync.dma_start(out=fw1s[:], in_=fw1_d[:].transpose([1, 0, 2]))
            fw2s = cp.tile([128, 8, FO], dt)
            nc.sync.dma_start(out=fw2s[:], in_=fw2_d[:].transpose([1, 0, 2]))
            drain(gcn_gen(0))
            for blk in range(nblk):
                if blk + 2 < nblk:
                    prologue(blk + 2)
                drain(head_gen(blk),
                      gcn_gen(blk + 1) if blk + 1 < nblk else None,
                      pool_head_gen() if blk == nblk - 1 else None)



    nc.compile()
    return nc


def _np_dt(dt):
    return ml_dtypes.bfloat16 if dt == mybir.dt.bfloat16 else np.float32


def host_prep(x, src, dst, W1, b1, W2, b2, W3, b3, fW1, fb1, fW2, fb2,
              dt=mybir.dt.bfloat16):
    """Host-side index bookkeeping + per-core sharding. Returns in_maps."""
    ndt = _np_dt(dt)
    src = np.asarray(src).astype(np.int64)
    dst = np.asarray(dst).astype(np.int64)

    # Natural per-graph adjacency counts adj[g, d, s] = #(edges s->d) + I.
    g = src >> 7
    cell = (g << 14) | ((dst & 127) << 7) | (src & 127)
    cnt = np.bincount(cell, minlength=B * NPG * NPG)
    diag = ((np.arange(B, dtype=np.int64) << 14)[:, None]
            + (np.arange(NPG, dtype=np.int64) * (NPG + 1))[None, :]).ravel()
    cnt[diag] += 1
    assert cnt.max() < 256, "adjacency count overflow"
    adj = cnt.astype(ndt).reshape(B, NPG, NPG)

    x = np.asarray(x, dtype=np.float32).astype(ndt).reshape(B, NPG, F)
    common = dict(
        ident=np.eye(NPG, dtype=np.float32).astype(ndt),
        w1=np.asarray(W1, np.float32).astype(ndt),
        w2=np.asarray(W2, np.float32).astype(ndt),
        w3=np.asarray(W3, np.float32).astype(ndt),
        fw1=np.asarray(fW1, np.float32).astype(ndt).reshape(2, 128, FH),
        fw2=np.asarray(fW2, np.float32).astype(ndt).reshape(8, 128, FO),
        b1=np.asarray(b1, np.float32),
        b2x=np.tile(np.asarray(b2, np.float32), BG),
        b3=np.asarray(b3, np.float32).reshape(2, 128),
        fb1=np.asarray(fb1, np.float32).reshape(8, 128),
        fb2=np.asarray(fb2, np.float32),
    )
    in_maps = []
    for c in range(NCORES):
        in_maps.append(dict(
            x=np.ascontiguousarray(x[c * GPC:(c + 1) * GPC]),
            adj=np.ascontiguousarray(adj[c * GPC:(c + 1) * GPC]),
            **common,
        ))
    return in_maps


_compiled = {}


def _get_program(dt):
    key = str(dt)
    if key not in _compiled:
        _compiled[key] = build_program(dt=dt)
    return _compiled[key]


def kernel(x, src, dst, batch, W1, b1, W2, b2, W3, b3, fW1, fb1, fW2, fb2,
           dt=mybir.dt.bfloat16):
    # `batch` is the deterministic repeat(arange(B), NPG) — structure hardcoded.
    in_maps = host_prep(x, src, dst, W1, b1, W2, b2, W3, b3, fW1, fb1, fW2, fb2,
                        dt=dt)
    nc = _get_program(dt)
    res = run_bass_kernel_spmd(nc, in_maps, list(range(NCORES)))
    outs = res.results
    out_t = np.concatenate([r["out_t"] for r in outs], axis=0)    # [B, o, n]
    p = np.concatenate([r["out_pool"] for r in outs], axis=0)
    out = np.ascontiguousarray(out_t.transpose(0, 2, 1))          # [B, n, o]
    return out, p.reshape(B, FO)
